# revision 20
# baseline (speedup 1.0000x reference)
"""Trainium2 Bass kernel for an AttnBlock (GroupNorm -> single-head attention
-> out-proj -> residual) on x[2, 512, 64, 64].

Sharding: 8 cores = batch(2) x query-chunk(4).  Each core receives its batch's
full x with its own 1024 query columns permuted to the front (GroupNorm stats
and softmax sums over spatial positions are permutation invariant), computes
GN for all 4096 positions, and attention for its 1024 queries.

Weight algebra is folded HOST-side (weights-only transforms, O(C^2)):
  M   = wq^T wk * c^-0.5      so scores[j,i] = hn_j^T M hn_i (+ t[j] terms)
  W2  = wo wv                 so out = W2 (hn A) / den + b2 + x
  b2  = wo bv + bo
The device computes, all in fp8(e4m3) DoubleRow matmuls with fp32 PSUM:
  q2   = M^T hn               (own 1024 queries)
  P2T  = (W2 hn)^T            [j, o] orientation, one GEMM, no transposes
  s    = hn^T q2 ; a = exp(s) ; den = sum_j a
  out  = (P2T^T a) / den + x  (attention + out-proj fused in ONE GEMM)
"""

import numpy as np
import ml_dtypes

import concourse.bass as bass
import concourse.tile as tile
from concourse import mybir

P = 128
C = 512
N = 4096
NQ = 1024          # queries per core
CCN = 4            # channel chunks of 128
NB = 8             # n chunks of 512
JCN = 32           # j chunks of 128
UCN = 16           # j chunk pairs (DoubleRow)
IBN = 2            # i blocks of 512 per core
SCALE = float(C) ** -0.5
EPS = 1e-6
GROUP = 16         # channels per group

# fp8 scale plan (see module docstring algebra):
SM = 1024.0        # M8 = fp8(M * SM)
SQ = 64.0          # q28 = fp8(q2 * SQ) = fp8(q_psum * SQ/SM)
SW2 = 512.0        # W2T8 = fp8(W2^T * SW2)
SPP = 16.0         # P2T8 = fp8(P2T * SPP) = fp8(p_psum * SPP/SW2)
SU = 64.0          # u8 = fp8(u * SU) for the t-vector path

F32 = mybir.dt.float32
BF16 = mybir.dt.bfloat16
FP8 = mybir.dt.float8e4
AF = mybir.ActivationFunctionType
ALU = mybir.AluOpType
DR = mybir.MatmulPerfMode.DoubleRow
BF16NP = ml_dtypes.bfloat16
FP8NP = ml_dtypes.float8_e4m3

_WAIT_LIMIT = 1


def _split_excess_waits(nc):
    """This walrus build rejects multi-wait sync on one instruction.  Move
    excess waits onto same-engine NoOps inserted just before the offending
    instruction; engine queues (and the SP DMA-trigger stream) are FIFO, so
    semantics are preserved."""
    counter = 0
    for f in nc.m.functions:
        for bb in f.blocks:
            insts = bb.instructions
            out = []
            for ins in insts:
                si = ins.sync_info
                waits = list(si.on_wait) if si and si.on_wait else []
                if len(waits) > _WAIT_LIMIT:
                    si.on_wait = waits[-_WAIT_LIMIT:]
                    extra = waits[:-_WAIT_LIMIT]
                    for i in range(0, len(extra), _WAIT_LIMIT):
                        nop = mybir.InstNoOp(
                            name=f"I-wsplit-{counter}", ins=[], outs=[])
                        counter += 1
                        nop.engine = ins.engine
                        nop.sync_info = mybir.SyncInfo(
                            on_wait=extra[i:i + _WAIT_LIMIT], on_update=[])
                        out.append(nop)
                out.append(ins)
            insts[:] = out


def build_program(with_t=False, with_b2=False, split_waits=True):
    nc = bass.Bass("TRN2", target_bir_lowering=False, debug=False)

    xp = nc.dram_tensor("xp", [C, N], BF16, kind="ExternalInput").ap()
    m8_d = nc.dram_tensor("m8", [C, C], FP8, kind="ExternalInput").ap()
    w2t8_d = nc.dram_tensor("w2t8", [C, C], FP8, kind="ExternalInput").ap()
    u8_d = nc.dram_tensor("u8", [C], FP8, kind="ExternalInput").ap()
    b2_d = nc.dram_tensor("b2", [C], F32, kind="ExternalInput").ap()
    gam_d = nc.dram_tensor("gamma", [C], F32, kind="ExternalInput").ap()
    bet_d = nc.dram_tensor("beta", [C], F32, kind="ExternalInput").ap()
    sel_d = nc.dram_tensor("sel", [P, 8], F32, kind="ExternalInput").ap()
    bsel_d = nc.dram_tensor("bsel", [8, P], F32, kind="ExternalInput").ap()
    ones8_d = nc.dram_tensor("ones8", [P, 2 * P], FP8, kind="ExternalInput").ap()
    out_d = nc.dram_tensor("out", [C, NQ], BF16, kind="ExternalOutput").ap()

    xv = xp.rearrange("(cc p) n -> p cc n", p=P)
    m8v = m8_d.rearrange("(cc p) o -> p cc o", p=P)
    w2v = w2t8_d.rearrange("(cc p) o -> p cc o", p=P)
    ov = out_d.rearrange("(oc p) n -> p oc n", p=P)

    with tile.TileContext(nc) as tc:
        _emit(nc, tc, xv, ov, m8v, w2v,
              dict(u8=u8_d, b2=b2_d, gam=gam_d, bet=bet_d),
              dict(sel=sel_d, bsel=bsel_d, ones8=ones8_d),
              with_t=with_t, with_b2=with_b2)
    if split_waits:
        _split_excess_waits(nc)
    return nc


def _emit(nc, tc, xv, ov, m8v, w2v, vd, cd, with_t, with_b2):
    from contextlib import ExitStack
    ctx = ExitStack()
    with ctx:
        const = ctx.enter_context(tc.tile_pool(name="const", bufs=1))
        persist = ctx.enter_context(tc.tile_pool(name="persist", bufs=1))
        evac = ctx.enter_context(tc.tile_pool(name="evac", bufs=2))
        dram = ctx.enter_context(tc.tile_pool(name="dram", bufs=1, space="DRAM"))

        # ---- constants / small vectors ----
        sel = const.tile([P, 8], F32)
        nc.sync.dma_start(sel[:], cd["sel"][:])
        bsel = const.tile([8, P], F32)
        nc.sync.dma_start(bsel[:], cd["bsel"][:])
        ones8 = const.tile([P, 2, P], FP8)
        nc.sync.dma_start(ones8[:], cd["ones8"].rearrange("p (a b) -> p a b", a=2))

        def vec128(name, src):
            t = const.tile([P, CCN], F32, name=name)
            nc.sync.dma_start(t[:], src.rearrange("(cc p) -> p cc", p=P))
            return t

        gam_sb = vec128("gam_sb", vd["gam"])
        bet_sb = vec128("bet_sb", vd["bet"])
        b2_sb = None
        if with_b2:
            b2_sb = vec128("b2_sb", vd["b2"])
        ut8 = const.tile([P, CCN], FP8)
        if with_t:
            nc.sync.dma_start(ut8[:], vd["u8"].rearrange("(cc p) -> p cc", p=P))

        M8 = persist.tile([P, CCN, C], FP8)      # M[c, c'] * SM
        W2T8 = persist.tile([P, CCN, C], FP8)    # W2^T[c, o] * SW2

        hn = persist.tile([P, CCN, N], FP8)      # GN(x), fp8
        q28 = persist.tile([P, CCN, NQ], FP8)    # q2 * SQ
        P2T8 = persist.tile([P, JCN, C], FP8)    # (W2 hn)^T * SPP, [j, o]
        t_part = const.tile([P, JCN], F32)       # t[j] laid out [p, jc]
        A_sb = const.tile([P, CCN], F32)
        B_sb = const.tile([P, CCN], F32)
        bnbuf = const.tile([P, CCN, NB, 6], F32)
        mv = const.tile([P, CCN, 2], F32)

        # ---- x load (resident) + GN stats chasing the DMA pieces ----
        # Ramped piece sizes: the 16 HWDGE queues run in parallel, so the
        # first wave completes at (piece bytes)/(per-queue BW).  Small leading
        # pieces let bn_stats start within a few us instead of ~16us.
        xpool = tc.alloc_tile_pool(name="xres", bufs=1)
        xfull = xpool.tile([P, CCN, N], BF16, name="xfull", tag="x")
        # first two windows in 256-col halves so bn_stats starts early, then
        # 512-col pieces; M8/W2T8 queue behind x (needed ~15us later).
        for nb in range(2):
            for h in range(2):
                for cc in range(CCN):
                    c0 = nb * 512 + h * 256
                    nc.sync.dma_start(xfull[:, cc, c0:c0 + 256],
                                      xv[:, cc, c0:c0 + 256])
        for nb in range(2, NB):
            for cc in range(CCN):
                nc.sync.dma_start(
                    xfull[:, cc, nb * 512:(nb + 1) * 512],
                    xv[:, cc, nb * 512:(nb + 1) * 512])
        for cc in range(CCN):
            nc.sync.dma_start(M8[:, cc, :], m8v[:, cc, :])
            nc.sync.dma_start(W2T8[:, cc, :], w2v[:, cc, :])
        for nb in range(NB):
            for cc in range(CCN):
                nc.vector.bn_stats(bnbuf[:, cc, nb, :],
                                   xfull[:, cc, nb * 512:(nb + 1) * 512])

        pearly = tc.alloc_tile_pool(name="pearly", bufs=3, space="PSUM")

        # ---- GN stat aggregation -> per-channel A, B ----
        for cc in range(CCN):
            nc.vector.bn_aggr(mv[:, cc, :],
                              bnbuf[:, cc, :, :].rearrange("p a b -> p (a b)"))
        stats8 = const.tile([P, 8], F32)
        nc.vector.tensor_copy(stats8[:, 0:4], mv[:, :, 0])
        nc.vector.tensor_mul(stats8[:, 4:8], mv[:, :, 0], mv[:, :, 0])
        nc.vector.tensor_add(stats8[:, 4:8], stats8[:, 4:8], mv[:, :, 1])
        gs_ps = pearly.tile([8, 8], F32, tag="big")
        nc.tensor.matmul(gs_ps[:], sel[:], stats8[:], start=True, stop=True)
        gs_sb = const.tile([8, 8], F32)
        nc.vector.tensor_copy(gs_sb[:], gs_ps[:])
        gvar = const.tile([8, 4], F32)
        nc.vector.tensor_mul(gvar[:], gs_sb[:, 0:4], gs_sb[:, 0:4])
        nc.vector.tensor_sub(gvar[:], gs_sb[:, 4:8], gvar[:])
        nc.vector.tensor_scalar_add(gvar[:], gvar[:], EPS)
        gsq = const.tile([8, 4], F32)
        nc.scalar.sqrt(gsq[:], gvar[:])
        grs2 = const.tile([8, 8], F32)
        nc.vector.tensor_copy(grs2[:, 0:4], gs_sb[:, 0:4])
        nc.vector.reciprocal(grs2[:, 4:8], gsq[:])
        bc_ps = pearly.tile([P, 8], F32, tag="big")
        nc.tensor.matmul(bc_ps[:], bsel[:], grs2[:], start=True, stop=True)
        nc.vector.tensor_mul(A_sb[:], gam_sb[:], bc_ps[:, 4:8])
        nc.vector.scalar_tensor_tensor(B_sb[:], bc_ps[:, 0:4], -1.0, A_sb[:],
                                       op0=ALU.mult, op1=ALU.mult)
        nc.vector.tensor_add(B_sb[:], B_sb[:], bet_sb[:])

        # ---- apply GN -> hn (fp8), reading the resident x ----
        for nb in range(NB):
            for cc in range(CCN):
                dst = hn[:, cc, nb * 512:(nb + 1) * 512]
                xsl = xfull[:, cc, nb * 512:(nb + 1) * 512]
                if nb < 2:
                    eng = "v" if cc % 2 == 0 else "s"
                else:
                    eng = ("v", "s", "g", "g")[cc]
                if eng == "v":
                    nc.vector.tensor_scalar(dst, xsl,
                                            A_sb[:, cc:cc + 1],
                                            B_sb[:, cc:cc + 1],
                                            op0=ALU.mult, op1=ALU.add)
                elif eng == "s":
                    nc.scalar.activation(dst, xsl, AF.Identity,
                                         bias=B_sb[:, cc:cc + 1],
                                         scale=A_sb[:, cc:cc + 1])
                else:
                    nc.gpsimd.tensor_scalar(dst, xsl,
                                            A_sb[:, cc:cc + 1],
                                            B_sb[:, cc:cc + 1],
                                            op0=ALU.mult, op1=ALU.add)

        # ---- q2[c', i] = sum_c M[c, c'] hn[c, i]  (i in 0:1024), DR fp8 ----
        for cch in range(CCN):
            for ih in range(2):
                q_ps = pearly.tile([P, 512], F32, name="q_ps", tag="big")
                for h in range(2):
                    nc.tensor.matmul(q_ps[:],
                                     M8[:, 2 * h:2 * h + 2,
                                        cch * P:(cch + 1) * P],
                                     hn[:, 2 * h:2 * h + 2,
                                        ih * 512:(ih + 1) * 512],
                                     start=(h == 0), stop=(h == 1),
                                     perf_mode=DR)
                if (cch * 2 + ih) % 2 == 0:
                    nc.vector.tensor_scalar_mul(
                        q28[:, cch, ih * 512:(ih + 1) * 512], q_ps[:], SQ / SM)
                else:
                    nc.scalar.mul(q28[:, cch, ih * 512:(ih + 1) * 512],
                                  q_ps[:], SQ / SM)

        # ---- P2T[j, o] = sum_c hn[c, j] W2T[c, o], DR fp8 ----
        for jc in range(JCN):
            p_ps = pearly.tile([P, 512], F32, name="p_ps", tag="big")
            for h in range(2):
                nc.tensor.matmul(p_ps[:],
                                 hn[:, 2 * h:2 * h + 2, jc * P:(jc + 1) * P],
                                 W2T8[:, 2 * h:2 * h + 2, :],
                                 start=(h == 0), stop=(h == 1),
                                 perf_mode=DR, skip_group_check=True)
            if jc % 2 == 0:
                nc.vector.tensor_scalar_mul(P2T8[:, jc, :], p_ps[:], SPP / SW2)
            else:
                nc.scalar.mul(P2T8[:, jc, :], p_ps[:], SPP / SW2)

        if with_t:
            # t[n] = sum_c' u[c'] hn[c', n] -> DRAM bounce -> t_part[p, jc]
            t_dram = dram.tile([N], F32)
            for nb in range(NB):
                t_ps = pearly.tile([1, 512], F32, name="t_ps", tag="big")
                for h in range(2):
                    nc.tensor.matmul(t_ps[:], ut8[:, 2 * h:2 * h + 2],
                                     hn[:, 2 * h:2 * h + 2,
                                        nb * 512:(nb + 1) * 512],
                                     start=(h == 0), stop=(h == 1),
                                     perf_mode=DR, skip_group_check=True)
                t_ch = evac.tile([1, 512], F32, name="t_ch", tag="tch", bufs=1)
                nc.scalar.mul(t_ch[:], t_ps[:], 1.0 / SU)
                nc.sync.dma_start(t_dram[nb * 512:(nb + 1) * 512], t_ch[:])
            nc.sync.dma_start(t_part[:], t_dram.rearrange("(jc p) -> p jc", p=P))

        pearly.release()

        # ---- attention ----
        # scores(ib0), scores(ib1) with exp -> fp8 aT pair tiles; then per
        # i-block: softmax denominator summed ON PE (ones8 DoubleRow against
        # each aT pair tile, accumulating a [1, 512] PSUM), the fused
        # (attention x out-proj) GEMM accumulating P2T^T a over all 32 j
        # chunks, and evacuation normalized by 1/(SPP*den).
        aTpool = tc.alloc_tile_pool(name="aT", bufs=34)
        patt = tc.alloc_tile_pool(name="patt", bufs=1, space="PSUM")
        aTs = {}
        for ib in range(IBN):
            i0 = ib * 512
            for jc in range(JCN):
                u, par = divmod(jc, 2)
                s_ps = patt.tile([P, 512], F32, name="s_ps", tag="s", bufs=3)
                for h in range(2):
                    nc.tensor.matmul(s_ps[:],
                                     hn[:, 2 * h:2 * h + 2, jc * P:(jc + 1) * P],
                                     q28[:, 2 * h:2 * h + 2, i0:i0 + 512],
                                     start=(h == 0), stop=(h == 1),
                                     perf_mode=DR)
                if par == 0:
                    aT_t = aTpool.tile([P, 2, 512], FP8, name="aT_t", tag="aT",
                                       bufs=34)
                    aTs[ib, u] = aT_t
                aT_t = aTs[ib, u]
                if with_t:
                    nc.scalar.activation(aT_t[:, par, :], s_ps[:], AF.Exp,
                                         bias=t_part[:, jc:jc + 1],
                                         scale=1.0 / SQ)
                else:
                    nc.scalar.activation(aT_t[:, par, :], s_ps[:], AF.Exp,
                                         scale=1.0 / SQ)

        for ib in range(IBN):
            # den[i] = sum_j a[j, i] on PE, broadcast to all 128 partitions
            # by the all-ones stationary; reciprocal on DVE overlaps the
            # AVproj matmuls that follow.
            den_ps = patt.tile([P, 512], F32, name=f"den_ps{ib}", tag="s",
                               bufs=3)
            for u in range(UCN):
                nc.tensor.matmul(den_ps[:], ones8[:], aTs[ib, u][:],
                                 start=(u == 0), stop=(u == UCN - 1),
                                 perf_mode=DR, skip_group_check=True)
            recip = const.tile([P, 512], BF16, name=f"recip{ib}")
            with nc.allow_low_precision(reason="bf16 1/denominator is ample"):
                nc.vector.reciprocal(recip[:], den_ps[:])

            # fused (attention x out-proj): op[o, i] = sum_j P2T[j, o] a[j, i]
            i0 = ib * 512
            for oc in range(CCN):
                op_ps = patt.tile([P, 512], F32, name=f"op_ps{oc}", tag="av",
                                  bufs=4)
                for u in range(UCN):
                    nc.tensor.matmul(op_ps[:],
                                     P2T8[:, 2 * u:2 * u + 2,
                                          oc * P:(oc + 1) * P],
                                     aTs[ib, u][:],
                                     start=(u == 0), stop=(u == UCN - 1),
                                     perf_mode=DR, skip_group_check=True)
                osb = evac.tile([P, 512], BF16, name="osb", tag="osb")
                nc.vector.scalar_tensor_tensor(osb[:], op_ps[:],
                                               1.0 / SPP, recip[:],
                                               op0=ALU.mult, op1=ALU.mult)
                xr = xfull[:, oc, i0:i0 + 512]
                if with_b2:
                    nc.vector.scalar_tensor_tensor(osb[:], osb[:],
                                                   b2_sb[:, oc:oc + 1], xr,
                                                   op0=ALU.add, op1=ALU.add)
                elif oc % 2 == 0:
                    nc.gpsimd.tensor_add(osb[:], osb[:], xr)
                else:
                    nc.vector.tensor_add(osb[:], osb[:], xr)
                nc.sync.dma_start(ov[:, oc, i0:i0 + 256], osb[:, 0:256])
                nc.sync.dma_start(ov[:, oc, i0 + 256:i0 + 512],
                                  osb[:, 256:512])

        aTpool.release()
        patt.release()
        xpool.release()


# ---------------- host side ----------------

_CACHED = {}


def _get_nc(with_t, with_b2):
    key = (with_t, with_b2)
    if key not in _CACHED:
        _CACHED[key] = build_program(with_t=with_t, with_b2=with_b2)
    return _CACHED[key]


def _host_constants():
    p = np.arange(P)
    sel = np.zeros((P, 8), np.float32)
    sel[p, p // GROUP] = 1.0 / GROUP
    bsel = np.zeros((8, P), np.float32)
    bsel[p // GROUP, p] = 1.0
    ones8 = np.ones((P, 2 * P), dtype=FP8NP)
    return dict(sel=sel, bsel=bsel, ones8=ones8)


def _host_weights(wq, bq, wk, wv, bv, wo, bo):
    """Weights-only folds (input-independent): M, W2, b2, u."""
    wq = np.asarray(wq, np.float32)
    wk = np.asarray(wk, np.float32)
    wv = np.asarray(wv, np.float32)
    wo = np.asarray(wo, np.float32)
    M = (wq.T @ wk) * SCALE
    W2 = wo @ wv
    b2 = wo @ np.asarray(bv, np.float32) + np.asarray(bo, np.float32)
    u = (wk.T @ np.asarray(bq, np.float32)) * SCALE
    return (np.ascontiguousarray((M * SM).astype(FP8NP)),
            np.ascontiguousarray((W2.T * SW2).astype(FP8NP)),
            b2.astype(np.float32),
            (u * SU).astype(FP8NP))


def kernel(x, gn_scale, gn_bias, wq, bq, wk, bk, wv, bv, wo, bo):
    from concourse.bass_utils import run_bass_kernel_spmd

    m8, w2t8, b2, u8 = _host_weights(wq, bq, wk, wv, bv, wo, bo)
    with_t = bool(np.any(np.asarray(bq, np.float32) != 0))
    with_b2 = bool(np.any(b2 != 0))
    nc = _get_nc(with_t, with_b2)
    consts = _host_constants()
    xr = np.ascontiguousarray(
        np.asarray(x, np.float32).reshape(2, C, N).astype(BF16NP))
    shared = dict(
        m8=m8, w2t8=w2t8, b2=b2, u8=u8,
        gamma=np.asarray(gn_scale, np.float32),
        beta=np.asarray(gn_bias, np.float32),
        **consts,
    )
    in_maps = []
    for core in range(8):
        b, qc = divmod(core, 4)
        perm_x = np.concatenate(
            [xr[b][:, qc * NQ:(qc + 1) * NQ],
             np.delete(xr[b], np.s_[qc * NQ:(qc + 1) * NQ], axis=1)], axis=1)
        in_maps.append({"xp": np.ascontiguousarray(perm_x), **shared})

    res = run_bass_kernel_spmd(nc, in_maps, core_ids=list(range(8)))
    y = np.empty((2, C, N), np.float32)
    for core in range(8):
        b, qc = divmod(core, 4)
        y[b][:, qc * NQ:(qc + 1) * NQ] = res.results[core]["out"].astype(
            np.float32)
    return y.reshape(2, C, 64, 64)
